# revision 1
# baseline (speedup 1.0000x reference)
"""Trainium2 Bass kernel for nn_Decoder (gnn_message_passing).

12-step LSTM decoder with (N,N) pairwise pooling, N=512 agents, sharded over
8 NeuronCores by agent rows (64 agents/core).

Key algebra: the pairwise MLP first layer collapses:
  feat[i,j] = [corr@W_se | h[j] | h[i]],  corr[i,j] = pos[i]-pos[j]
  feat @ W1 + b1 = P[i] + Q[j]
  P[i] = pos[i]@A + h[i]@W1[40:48] + (b1 + b_se@W1[0:32])
  Q[j] = h[j]@W1[32:40] - pos[j]@A,   A = W_se @ W1[0:32]   (64-dim)
so per step each core computes its own P (64 agents) and Q-block, all-gathers
Q (the only cross-core exchange), then for each pair (i,j):
  ph = relu( relu(P[i]+Q[j]) @ W2 + b2 );  ctx[i] = masked-max_j ph
The masked max folds the neighbor mask into the PE via an accumulated
"mask matmul" adding BIG*nei[i,j] to PSUM, then a plain reduce_max and a
final relu(x - BIG + b2) epilogue.

Pooling layout per core/step: agents il = 16b + 8q + 4p + t
  (t = PSUM tile 0..3, b = 32-partition strip 0..3, q = strip half, p = parity)
PSUM tile t partition u = 32b + 16q + 2f + p  (f = feature 0..7), dense via
two zero-padded M=32 matmuls accumulated per strip.
"""
import numpy as np
from contextlib import ExitStack

import concourse.bass as bass
import concourse.bacc as bacc
import concourse.mybir as mybir
from concourse import tile
from concourse.bass_utils import run_bass_kernel_spmd

F32 = mybir.dt.float32
BF16 = mybir.dt.bfloat16
I32 = mybir.dt.int32

N = 512
R = 8            # cores
NA = N // R      # agents per core = 64
NSTEPS = 12
D = 64           # pooling hidden dim
BIG = 512.0

AluOp = mybir.AluOpType
Act = mybir.ActivationFunctionType


# ---------------------------------------------------------------------------
# host-side constant packing
# ---------------------------------------------------------------------------

def build_constants(W_in, b_in, W_ih, W_hh, b_ih, b_hh, W_m, b_m, W_v, b_v,
                    W_zh, b_zh, W_se, b_se, W1, b1, W2, b2):
    c = {}
    A = W_se @ W1[0:32]                      # [2, 64]
    c["A_T"] = np.ascontiguousarray(A)       # lhsT [2, 64] for +a
    c["negA_T"] = np.ascontiguousarray(-A)
    c["W1u_T"] = np.ascontiguousarray(W1[32:40])   # [8, 64] lhsT for u (hj)
    c["W1v_T"] = np.ascontiguousarray(W1[40:48])   # [8, 64] lhsT for v (hi)
    c["b1p"] = (b1 + b_se @ W1[0:32]).reshape(64, 1).astype(np.float32)

    # pooling lhsT per strip-half q: [128, 32], col m = 16q + 2f + p
    for q in range(2):
        L = np.zeros((128, 32), dtype=np.float32)
        for p in range(2):
            for f in range(8):
                L[p * 64:(p + 1) * 64, 16 * q + 2 * f + p] = W2[:, f]
        c[f"Wpool_q{q}"] = L

    # mask lhsT [16, 128]: k = 8p + 2b + q -> BIG at u = 32b + 16q + 2f + p
    LM = np.zeros((16, 128), dtype=np.float32)
    for p in range(2):
        for b in range(4):
            for q in range(2):
                k = 4 * b + 2 * q + p
                for f in range(8):
                    LM[k, 32 * b + 16 * q + 2 * f + p] = BIG
    c["lhsT_mask"] = LM

    # ctx epilogue bias [128, 1]: b2[f] - BIG at u = 32b + 16q + 2f + p
    BC = np.zeros((128, 1), dtype=np.float32)
    for b in range(4):
        for q in range(2):
            for p in range(2):
                for f in range(8):
                    BC[32 * b + 16 * q + 2 * f + p, 0] = b2[f] - BIG
    c["bias_ctx"] = BC

    # x layer: x = relu(concat([ctx, prev, c, z]) @ W_in + b_in)
    c["Win_ctx"] = np.ascontiguousarray(W_in[0:8])    # [8, 16]
    c["Win_prev"] = np.ascontiguousarray(W_in[8:10])  # [2, 16]
    c["Win_c"] = np.ascontiguousarray(W_in[10:18])    # [8, 16]
    c["Win_z"] = np.ascontiguousarray(W_in[18:20])    # [2, 16]
    c["b_in"] = b_in.reshape(16, 1).astype(np.float32)

    # gates (torch order i,f,g,o in columns of W_ih/W_hh); psum layout:
    # g at partitions 0-7, i at 32-39, f at 64-71, o at 96-103
    Wih_all = np.zeros((16, 128), dtype=np.float32)
    Whh_all = np.zeros((8, 128), dtype=np.float32)
    for gi, g, base in ((2, "g", 0), (0, "i", 32), (1, "f", 64), (3, "o", 96)):
        sl = slice(8 * gi, 8 * gi + 8)
        Wih_all[:, base:base + 8] = W_ih[:, sl]
        Whh_all[:, base:base + 8] = W_hh[:, sl]
        c[f"bias_{g}"] = (b_ih[sl] + b_hh[sl]).reshape(8, 1).astype(np.float32)
    c["Wih_all"] = Wih_all
    c["Whh_all"] = Whh_all

    # mu/logvar: mu = [h[:, :4], ctx] @ W_m + b_m ; lv = [h[:, 4:], ctx] @ W_v + b_v
    Wmh = np.zeros((8, 2), dtype=np.float32); Wmh[0:4] = W_m[0:4]
    Wlh = np.zeros((8, 2), dtype=np.float32); Wlh[4:8] = W_v[0:4]
    c["Wm_h"] = Wmh
    c["Wv_h"] = Wlh
    c["Wm_ctx"] = np.ascontiguousarray(W_m[4:12])   # [8, 2]
    c["Wv_ctx"] = np.ascontiguousarray(W_v[4:12])   # [8, 2]
    c["b_m"] = b_m.reshape(2, 1).astype(np.float32)
    c["b_v"] = b_v.reshape(2, 1).astype(np.float32)
    c["half_b_v"] = (0.5 * b_v).reshape(2, 1).astype(np.float32)
    c["neg_half_b_v"] = (-0.5 * b_v).reshape(2, 1).astype(np.float32)

    c["Wzh_T"] = np.ascontiguousarray(W_zh)         # [2, 8]
    c["b_zh"] = b_zh.reshape(8, 1).astype(np.float32)
    return c


CONST_NAMES = [
    "A_T", "negA_T", "W1u_T", "W1v_T", "b1p", "Wpool_q0", "Wpool_q1",
    "lhsT_mask", "bias_ctx", "Win_ctx", "Win_prev", "Win_c", "Win_z", "b_in",
    "Wih_i", "Whh_i", "bias_i", "Wih_f", "Whh_f", "bias_f",
    "Wih_g", "Whh_g", "bias_g", "Wih_o", "Whh_o", "bias_o",
    "Wm_h", "Wv_h", "Wm_ctx", "Wv_ctx", "b_m", "b_v", "half_b_v",
    "Wzh_T", "b_zh",
]

PERCORE_NAMES = ["pT", "cT", "zT", "obslastT", "c0T", "epsT", "nei_own"]


# ---------------------------------------------------------------------------
# device program
# ---------------------------------------------------------------------------

def build_program(nsteps=NSTEPS):
    nc = bacc.Bacc("TRN2", target_bir_lowering=False, debug=False,
                   num_devices=R)

    io = {}
    # per-core inputs
    io["pT"] = nc.dram_tensor("pT", [2, NA], F32, kind="ExternalInput")
    io["cT"] = nc.dram_tensor("cT", [8, NA], F32, kind="ExternalInput")
    io["zT"] = nc.dram_tensor("zT", [2, NA], F32, kind="ExternalInput")
    io["obslastT"] = nc.dram_tensor("obslastT", [2, NA], F32, kind="ExternalInput")
    io["c0T"] = nc.dram_tensor("c0T", [8, NA], F32, kind="ExternalInput")
    io["epsT"] = nc.dram_tensor("epsT", [2, nsteps * NA], F32, kind="ExternalInput")
    io["nei_own"] = nc.dram_tensor("nei_own", [nsteps, NA, N], I32, kind="ExternalInput")
    # constants
    shapes = {
        "A_T": [2, D], "negA_T": [2, D], "W1u_T": [8, D], "W1v_T": [8, D],
        "b1p": [D, 1], "Wpool_q0": [128, 32], "Wpool_q1": [128, 32],
        "lhsT_mask": [16, 128], "bias_ctx": [128, 1],
        "Win_ctx": [8, 16], "Win_prev": [2, 16], "Win_c": [8, 16],
        "Win_z": [2, 16], "b_in": [16, 1],
        "Wm_h": [8, 2], "Wv_h": [8, 2], "Wm_ctx": [8, 2], "Wv_ctx": [8, 2],
        "b_m": [2, 1], "b_v": [2, 1], "half_b_v": [2, 1], "neg_half_b_v": [2, 1],
        "Wzh_T": [2, 8], "b_zh": [8, 1],
    }
    shapes["Wih_all"] = [16, 128]
    shapes["Whh_all"] = [8, 128]
    for g in ("i", "f", "g", "o"):
        shapes[f"bias_{g}"] = [8, 1]
    for name, shp in shapes.items():
        io[name] = nc.dram_tensor(name, shp, F32, kind="ExternalInput")

    # outputs [nsteps, NA, 2]
    o_pos = nc.dram_tensor("out_positions", [2, nsteps, NA], F32, kind="ExternalOutput")
    o_mu = nc.dram_tensor("out_means", [2, nsteps, NA], F32, kind="ExternalOutput")
    o_lv = nc.dram_tensor("out_logvars", [2, nsteps, NA], F32, kind="ExternalOutput")

    with tile.TileContext(nc) as tc, ExitStack() as ctx:
        sb1 = ctx.enter_context(tc.tile_pool(name="consts", bufs=1))
        sbs = ctx.enter_context(tc.tile_pool(name="state", bufs=2))
        sbw = ctx.enter_context(tc.tile_pool(name="work", bufs=3))
        sbh = ctx.enter_context(tc.tile_pool(name="h1p", bufs=6))
        sbm = ctx.enter_context(tc.tile_pool(name="maskp", bufs=10))
        pp = ctx.enter_context(tc.tile_pool(name="poolps", bufs=4, space="PSUM"))
        sp = ctx.enter_context(tc.tile_pool(name="smallps", bufs=1, space="PSUM"))
        dr = ctx.enter_context(tc.tile_pool(name="dram", bufs=2, space="DRAM"))

        # ---- load constants / inputs into SBUF ----
        cst = {}
        bf16_consts = {"Wpool_q0", "Wpool_q1", "lhsT_mask"}
        for name, shp in shapes.items():
            if name in bf16_consts:
                t_ = sb1.tile(shp, BF16, tag=name)
                nc.gpsimd.dma_start(t_[:, :], io[name][:, :])
            else:
                t_ = sb1.tile(shp, F32, tag=name)
                nc.sync.dma_start(t_[:, :], io[name][:, :])
            cst[name] = t_

        cT = sb1.tile([8, NA], F32, tag="cT")
        nc.sync.dma_start(cT[:, :], io["cT"][:, :])
        zT = sb1.tile([2, NA], F32, tag="zT")
        nc.sync.dma_start(zT[:, :], io["zT"][:, :])
        pT = sb1.tile([2, NA], F32, tag="pT")
        nc.sync.dma_start(pT[:, :], io["pT"][:, :])
        epsT = sb1.tile([2, nsteps * NA], F32, tag="epsT")
        nc.sync.dma_start(epsT[:, :], io["epsT"][:, :])

        # persistent state
        posT = sb1.tile([2, NA], F32, tag="posT")
        nc.sync.dma_start(posT[:, :], io["obslastT"][:, :])
        clT = sb1.tile([8, NA], F32, tag="clT")
        nc.sync.dma_start(clT[:, :], io["c0T"][:, :])

        # h0 = z @ W_zh + b_zh
        ps_h0 = sp.tile([8, NA], F32, tag="ps_gate")
        nc.tensor.matmul(ps_h0[:, :], cst["Wzh_T"][:, :], zT[:, :],
                         start=True, stop=True)
        hT = sb1.tile([8, NA], F32, tag="hT")
        nc.scalar.activation(hT[:, :], ps_h0[:, :], Act.Identity,
                             bias=cst["b_zh"][:, :])

        ctxT = sb1.tile([8, NA], F32, tag="ctxT")
        nc.vector.memset(ctxT[:, :], 0.0)

        # output accumulators [2, nsteps*NA]
        ob_pos = sb1.tile([2, nsteps * NA], F32, tag="ob_pos")
        ob_mu = sb1.tile([2, nsteps * NA], F32, tag="ob_mu")
        ob_lv = sb1.tile([2, nsteps * NA], F32, tag="ob_lv")

        for s in range(nsteps):
            prevT = pT[:, :] if s == 0 else ob_pos[:, (s - 1) * NA: s * NA]

            # ---------------- LSTM ----------------
            # x = relu([ctx, prev, c, z] @ W_in + b_in)   -> xT [16, NA]
            ps_x = sp.tile([16, NA], F32, tag="ps_x")
            nc.tensor.matmul(ps_x[:, :], cst["Win_c"][:, :], cT[:, :],
                             start=True, stop=False)
            nc.tensor.matmul(ps_x[:, :], cst["Win_z"][:, :], zT[:, :],
                             start=False, stop=False)
            nc.tensor.matmul(ps_x[:, :], cst["Win_ctx"][:, :], ctxT[:, :],
                             start=False, stop=False)
            nc.tensor.matmul(ps_x[:, :], cst["Win_prev"][:, :], prevT,
                             start=False, stop=True)
            xT = sbw.tile([16, NA], F32, tag="xT")
            nc.scalar.activation(xT[:, :], ps_x[:, :], Act.Relu,
                                 bias=cst["b_in"][:, :])

            # gates: partition-blocked psum [128, NA]: g@0, i@32, f@64, o@96
            ps_g = sp.tile([128, NA], F32, tag="ps_gate")
            nc.tensor.matmul(ps_g[:, :], cst["Wih_all"][:, :], xT[:, :],
                             start=True, stop=False)
            nc.tensor.matmul(ps_g[:, :], cst["Whh_all"][:, :], hT[:, :],
                             start=False, stop=True)
            tan_g = sbw.tile([8, NA], F32, tag="tan_g")
            nc.scalar.activation(tan_g[:, :], ps_g[0:8, :],
                                 Act.Tanh, bias=cst["bias_g"][:, :])
            sig_i = sbw.tile([8, NA], F32, tag="sig_i")
            nc.scalar.activation(sig_i[:, :], ps_g[32:40, :],
                                 Act.Sigmoid, bias=cst["bias_i"][:, :])
            sig_f = sbw.tile([8, NA], F32, tag="sig_f")
            nc.scalar.activation(sig_f[:, :], ps_g[64:72, :],
                                 Act.Sigmoid, bias=cst["bias_f"][:, :])
            sig_o = sbw.tile([8, NA], F32, tag="sig_o")
            nc.scalar.activation(sig_o[:, :], ps_g[96:104, :],
                                 Act.Sigmoid, bias=cst["bias_o"][:, :])

            # cl = sig_f*cl + sig_i*tanh(g) ; h = sig_o*tanh(cl)
            t1 = sbw.tile([8, NA], F32, tag="t1")
            nc.vector.tensor_mul(t1[:, :], sig_i[:, :], tan_g[:, :])
            t2 = sbw.tile([8, NA], F32, tag="t2")
            nc.vector.tensor_mul(t2[:, :], sig_f[:, :], clT[:, :])
            clT = sbs.tile([8, NA], F32, tag="clT_s")
            nc.vector.tensor_add(clT[:, :], t1[:, :], t2[:, :])
            tcl = sbw.tile([8, NA], F32, tag="tcl")
            nc.scalar.activation(tcl[:, :], clT[:, :], Act.Tanh)
            hT = sbs.tile([8, NA], F32, tag="hT_s")
            nc.vector.tensor_mul(hT[:, :], sig_o[:, :], tcl[:, :])

            # ---------------- Q/P build + AllGather ----------------
            ps_qp = sp.tile([D, 2 * NA], F32, tag="ps_qp")
            # Q = h @ W1u - pos @ A   (cols 0:NA); pos parts first (ready early)
            nc.tensor.matmul(ps_qp[:, 0:NA], cst["negA_T"][:, :], posT[:, :],
                             start=True, stop=False, skip_group_check=True)
            nc.tensor.matmul(ps_qp[:, 0:NA], cst["W1u_T"][:, :], hT[:, :],
                             start=False, stop=True, skip_group_check=True)
            # P = h @ W1v + pos @ A + b1p  (cols NA:2NA)
            nc.tensor.matmul(ps_qp[:, NA:2 * NA], cst["A_T"][:, :], posT[:, :],
                             start=True, stop=False, skip_group_check=True)
            nc.tensor.matmul(ps_qp[:, NA:2 * NA], cst["W1v_T"][:, :], hT[:, :],
                             start=False, stop=True, skip_group_check=True)

            qblk = sbw.tile([D, NA], BF16, tag="qblk")
            nc.scalar.copy(qblk[:, :], ps_qp[:, 0:NA])
            PT = sbw.tile([D, NA], F32, tag="PT")
            nc.scalar.activation(PT[:, :], ps_qp[:, NA:2 * NA], Act.Identity,
                                 bias=cst["b1p"][:, :])

            ag_in = dr.tile([D, NA], BF16, tag="ag_in")
            nc.sync.dma_start(ag_in[:, :], qblk[:, :])
            ag_out = dr.tile([R * D, NA], BF16, tag="ag_out")
            nc.gpsimd.collective_compute(
                "AllGather", AluOp.bypass,
                replica_groups=[list(range(R))],
                ins=[ag_in[:, :]],
                outs=[ag_out[:, :]],
            )
            # Qdup [128, 512]: partition (dup, d), free j = 64*rr + jl
            qdup = sbw.tile([128, N], BF16, tag="qdup")
            for half in range(2):
                nc.sync.dma_start(
                    qdup[half * D:(half + 1) * D, :].rearrange(
                        "d (rr jl) -> d rr jl", rr=R, jl=NA),
                    ag_out.rearrange("(rr d) jl -> d rr jl", rr=R, d=D),
                )

            # Pdup [128, 32]: col pk = 8b + 4q + t ; lower = P[:, il(p=0)],
            # upper = P[:, il(p=1)], il = 16b + 8q + 4p + t
            pdup = sbw.tile([128, 32], F32, tag="pdup")
            for half in range(2):
                for q in range(2):
                    nc.sync.dma_start(
                        pdup.rearrange("P (b q t) -> P q b t", b=4, q=2, t=4)[
                            half * D:(half + 1) * D, q],
                        PT.rearrange("d (b q pt) -> d q b pt", b=4, q=2, pt=8)[
                            :, q, :, 4 * half: 4 * half + 4],
                    )

            # ---------------- pooling ----------------
            ctx_mx = sbw.tile([128, 4], F32, tag="ctx_mx")
            for t in range(4):
                # mask rhs [16, N] f32: rows k = 8p + (2b+q)
                mk = sbm.tile([16, N], BF16, tag="mask")
                nc.gpsimd.dma_start(mk[:, :], io["nei_own"][s, t::4, :])
                pt_ = pp.tile([128, N], F32, tag="poolps")
                # mask-MM opens the accumulation: it only needs the (prefetched)
                # mask DMA, so the PE can run it during the AllGather window
                nc.tensor.matmul(pt_[:, :], cst["lhsT_mask"][:, :], mk[:, :],
                                 start=True, stop=False, skip_group_check=True)
                for b in range(4):
                    for q in range(2):
                        pk = 8 * b + 4 * q + t
                        h1 = sbh.tile([128, N], BF16, tag="h1")
                        if pk % 3 == 2:
                            nc.scalar.activation(
                                h1[:, :], qdup[:, :], Act.Relu,
                                bias=pdup[:, pk:pk + 1])
                        else:
                            nc.vector.tensor_scalar(
                                h1[:, :], qdup[:, :], pdup[:, pk:pk + 1], 0.0,
                                op0=AluOp.add, op1=AluOp.max)
                        nc.tensor.matmul(
                            pt_[32 * b:32 * b + 32, :],
                            cst[f"Wpool_q{q}"][:, :], h1[:, :],
                            start=False, stop=(b == 3 and q == 1),
                            skip_group_check=True,
                            tile_position=(0, 32 * b))
                nc.vector.tensor_reduce(
                    ctx_mx[:, t:t + 1], pt_[:, :], axis=mybir.AxisListType.X,
                    op=AluOp.max)

            ctx_all = sbw.tile([128, 4], F32, tag="ctx_all")
            nc.scalar.activation(ctx_all[:, :], ctx_mx[:, :],
                                 Act.Relu, bias=cst["bias_ctx"][:, :])
            ctx_d = dr.tile([128, 4], F32, tag="ctx_d")
            nc.sync.dma_start(ctx_d[:, :], ctx_all[:, :])
            ctxT = sbs.tile([8, NA], F32, tag="ctxT_s")
            for q in range(2):
                for p in range(2):
                    nc.sync.dma_start(
                        ctxT.rearrange("f (b q p2 t) -> q p2 f b t",
                                       b=4, q=2, p2=2, t=4)[q, p],
                        ctx_d.rearrange("(b q f p2) t -> q p2 f b t",
                                        b=4, q=2, f=8, p2=2)[q, p],
                    )

            # ---------------- outputs ----------------
            ps_mv = sp.tile([2, 2 * NA], F32, tag="ps_mv")
            nc.tensor.matmul(ps_mv[:, 0:NA], cst["Wm_h"][:, :], hT[:, :],
                             start=True, stop=False, skip_group_check=True)
            nc.tensor.matmul(ps_mv[:, 0:NA], cst["Wm_ctx"][:, :], ctxT[:, :],
                             start=False, stop=True, skip_group_check=True)
            nc.tensor.matmul(ps_mv[:, NA:2 * NA], cst["Wv_h"][:, :], hT[:, :],
                             start=True, stop=False, skip_group_check=True)
            nc.tensor.matmul(ps_mv[:, NA:2 * NA], cst["Wv_ctx"][:, :],
                             ctxT[:, :], start=False, stop=True,
                             skip_group_check=True)
            sl = slice(s * NA, (s + 1) * NA)
            nc.scalar.activation(ob_mu[:, sl], ps_mv[:, 0:NA], Act.Identity,
                                 bias=cst["b_m"][:, :])
            nc.scalar.activation(ob_lv[:, sl], ps_mv[:, NA:2 * NA],
                                 Act.Identity, bias=cst["b_v"][:, :])
            # exp(y) = sigmoid(y)/sigmoid(-y), y = 0.5*logvar  (avoids the
            # exp act-table, which lives in a different function set than
            # sigmoid/tanh and would force 2 table reloads per step)
            ev_p = sbw.tile([2, NA], F32, tag="ev_p")
            nc.scalar.activation(ev_p[:, :], ps_mv[:, NA:2 * NA], Act.Sigmoid,
                                 bias=cst["half_b_v"][:, :], scale=0.5)
            ev_n = sbw.tile([2, NA], F32, tag="ev_n")
            nc.scalar.activation(ev_n[:, :], ps_mv[:, NA:2 * NA], Act.Sigmoid,
                                 bias=cst["neg_half_b_v"][:, :], scale=-0.5)
            ev_r = sbw.tile([2, NA], F32, tag="ev_r")
            nc.vector.reciprocal(ev_r[:, :], ev_n[:, :])
            ev = sbw.tile([2, NA], F32, tag="ev")
            nc.vector.tensor_mul(ev[:, :], ev_p[:, :], ev_r[:, :])
            pe = sbw.tile([2, NA], F32, tag="pe")
            nc.vector.tensor_mul(pe[:, :], epsT[:, sl], ev[:, :])
            nc.vector.tensor_add(ob_pos[:, sl], ob_mu[:, sl], pe[:, :])
            posT_new = sbs.tile([2, NA], F32, tag="posT_s")
            nc.vector.tensor_add(posT_new[:, :], posT[:, :], ob_pos[:, sl])
            posT = posT_new

        # final output DMAs: [2, (s, il)] -> dram [2, s, il]
        for ob, od in ((ob_pos, o_pos), (ob_mu, o_mu), (ob_lv, o_lv)):
            nc.sync.dma_start(
                od.rearrange("k s il -> k s il"),
                ob.rearrange("k (s il) -> k s il", s=nsteps, il=NA),
            )

    nc.compile()
    return nc


# ---------------------------------------------------------------------------
# host wrapper
# ---------------------------------------------------------------------------

def make_in_maps(inputs, nsteps=NSTEPS):
    inp = {k: np.asarray(v) for k, v in inputs.items()}
    cst = build_constants(
        inp["W_in"], inp["b_in"], inp["W_ih"], inp["W_hh"], inp["b_ih"],
        inp["b_hh"], inp["W_m"], inp["b_m"], inp["W_v"], inp["b_v"],
        inp["W_zh"], inp["b_zh"], inp["W_se"], inp["b_se"], inp["W1"],
        inp["b1"], inp["W2"], inp["b2"])

    in_maps = []
    for r in range(R):
        sl = slice(r * NA, (r + 1) * NA)
        m = dict(cst)
        m["pT"] = np.ascontiguousarray(inp["p"][sl].T)
        m["cT"] = np.ascontiguousarray(inp["c"][sl].T)
        m["zT"] = np.ascontiguousarray(inp["z"][sl].T)
        m["obslastT"] = np.ascontiguousarray(inp["obs_traj_pos"][-1, sl].T)
        m["c0T"] = np.ascontiguousarray(inp["c0_noise"][sl].T)
        m["epsT"] = np.ascontiguousarray(
            inp["eps"][:nsteps, sl, :].transpose(2, 0, 1).reshape(2, nsteps * NA))
        m["nei_own"] = np.ascontiguousarray(inp["nei_index"][:nsteps, sl, :])
        in_maps.append(m)
    return in_maps


_cached = {}


def kernel(**inputs):
    nsteps = NSTEPS
    if "nc" not in _cached:
        _cached["nc"] = build_program(nsteps)
    nc = _cached["nc"]
    in_maps = make_in_maps(inputs, nsteps)
    res = run_bass_kernel_spmd(nc, in_maps, list(range(R)))
    outs = res.results

    def unshard(name):
        per = [np.asarray(outs[r][name]).transpose(1, 2, 0) for r in range(R)]
        return np.concatenate(per, axis=1)

    return unshard("out_positions"), unshard("out_means"), unshard("out_logvars")

